# revision 13
# baseline (speedup 1.0000x reference)
# GAT layer kernel for 8 Trainium2 NeuronCores — v4 (multi-engine z-gen,
# software-pipelined front).
#
# Reference computation (per head h):
#   Wh = h @ W[h] + bW[h]                     [N, 64]
#   e[i,j] = LeakyReLU(a_l.Wh_i + a_r.Wh_j + bA, 0.2), masked, softmax over j
#   out[:, h*64:(h+1)*64] = elu(softmax(e) @ Wh)
#
# Algebraic restructure (no per-element transcendental): softmax rows are
# invariant to scaling by exp(el_i), so the unnormalized attention operand is
#   z[j,i] = mask[i,j] * max(F[j], F2[j]*Hn[i])
# with F = exp(er+bA), F2 = exp(0.2*(er+bA)), Hn = exp(-0.8*el).
#
# v4: the z elementwise work (the bottleneck) is spread over DVE, Pool and
# ACT via per-j-tile paths (both heads of a jt share one path so the mask
# multiply handles the head PAIR in one op with a broadcast mask AP):
#   AD: a2 = (hb*F2) max F on DVE (tensor_scalar dual, 4x mode) per head,
#       z = a2*mask on DVE (tensor_tensor 2x, [128,2,2048] pair)
#   AM: a2 on DVE, pair z = a2*mask on Pool (tensor_tensor Multiply)
#   CD: r = relu(F2*hb - F) on ACT (scale/bias per-partition pointers),
#       pair z_r = r*mask on DVE; the missing mask*F*Wh term is added by
#       extra PE matmuls (stationary = raw mask, moving = w1f = [F*Wh|F])
#   CM: r on ACT, pair z_r = r*mask on Pool, PE extra as in CD
# The steady matmul is reversed (stationary = z j-tile, moving = [Wh|1], 66
# cols -> 66 PE rows/matmul) and accumulators land in [i, o] layout.
#
# Front pipeline: el/Wh prep for j-tiles 16..31 depends on the second half of
# the ht DMA stream, so those preparations are EMITTED inside the steady loop
# (after the ht chunks land) to avoid head-of-line blocking every engine
# queue.  PSUM: 6 accumulator banks (head-1 g=2,3 are packed into the spare
# columns 66:132 of head-0's banks) + 1 bank for the deferred el projections
# + 1 rotating bank for Wh prep.
#
# Sharding: 8 cores = 4 head-pairs x 2 row-halves (2 heads x 2048 rows per
# core, attention over all 4096 columns); h/mask columns rolled per-core so
# own rows sit at j-tiles 0..15 (shared SPMD program).

import numpy as np
import ml_dtypes

N = 4096
F_IN = 512
F_OUT = 64
H = 8
NCORES = 8
RPC = 2048           # rows per core
KT = F_IN // 128     # 4 k-tiles
JT = N // 128        # 32 j-tiles
NJC = N // 512       # 8 ht DMA chunks
IB = RPC // 128      # 16 i-blocks
BF16 = ml_dtypes.bfloat16

# Per-jt path counts (both heads of a jt share the path).
# Pair costs: a2 2x593 DVE / relu 2x2077 ACT; mask-pair 2193 DVE / 8218 Pool;
# CD/CM add 2x16 extra PE matmuls (~0.9us).
_PJT = {
    'AD': 11,
    'AM': 6,
    'CD': 13,
    'CM': 2,
}

PLOOK = 2            # pairs of lookahead for phase-A emission


def _mk_path_table():
    order = []
    counts = dict(_PJT)
    assert sum(counts.values()) == 32
    acc = {k: 0.0 for k in counts}
    for _ in range(32):
        best = max(counts, key=lambda k: counts[k] / 32.0 * (len(order) + 1) - acc[k])
        order.append(best)
        acc[best] += 1.0
    return order


PATH_JT = _mk_path_table()

_prog_cache = {}


def _build_program():
    key = ("nc4",)
    if key in _prog_cache:
        return _prog_cache[key]
    from contextlib import ExitStack
    import concourse.tile as tile
    from concourse import bacc, mybir

    dt = mybir.dt
    f32, bf16, f16, f32r = dt.float32, dt.bfloat16, dt.float16, dt.float32r
    Alu = mybir.AluOpType
    Act = mybir.ActivationFunctionType

    nc = bacc.Bacc("TRN2", target_bir_lowering=False, debug=False,
                   num_devices=NCORES)

    ht_d = nc.dram_tensor("ht", [128, KT, N], bf16, kind="ExternalInput")
    wm_d = nc.dram_tensor("wm", [128, KT, 132], bf16, kind="ExternalInput")
    pack_d = nc.dram_tensor("pack", [128, 132], f32r, kind="ExternalInput")
    oh_d = nc.dram_tensor("oh", [16, 2048], f16, kind="ExternalInput")
    maskt_d = nc.dram_tensor("maskt", [JT // 2, 128, 2, RPC], bf16,
                             kind="ExternalInput")
    out_d = nc.dram_tensor("out", [2, 4, 128, 4 * F_OUT], f32,
                           kind="ExternalOutput")

    with tile.TileContext(nc) as tc, ExitStack() as ctx:
        singles = ctx.enter_context(tc.tile_pool(name="singles", bufs=1))
        psum = ctx.enter_context(tc.tile_pool(name="ps", bufs=8, space="PSUM"))
        mpool = ctx.enter_context(tc.tile_pool(name="mp", bufs=5))
        apool = ctx.enter_context(tc.tile_pool(name="ap", bufs=4))
        zpool = ctx.enter_context(tc.tile_pool(name="zp", bufs=5))
        htpool = ctx.enter_context(tc.tile_pool(name="ht", bufs=8))
        spool = ctx.enter_context(tc.tile_pool(name="sp", bufs=5))

        # ---- input DMA stream (one serial queue; order = priority) -------
        pack_sb = singles.tile([128, 132], f32r)
        nc.sync.dma_start(out=pack_sb, in_=pack_d.ap())
        ident_sb = pack_sb[:, 0:128]
        ba_sb = pack_sb[:, 128:132].bitcast(f32)

        wm_sb = singles.tile([128, KT, 132], bf16)
        nc.sync.dma_start(out=wm_sb, in_=wm_d.ap())

        mask_tiles = {}

        def prefetch_mask(jp, split=False):
            m_t = mpool.tile([128, 2, RPC], bf16, tag="m", name=f"mpre{jp}")
            if split:
                nc.sync.dma_start(out=m_t[:, 0, :], in_=maskt_d.ap()[jp, :, 0])
                nc.sync.dma_start(out=m_t[:, 1, :], in_=maskt_d.ap()[jp, :, 1])
            else:
                nc.sync.dma_start(out=m_t, in_=maskt_d.ap()[jp])
            mask_tiles[jp] = m_t

        def get_mask(jp):
            if jp not in mask_tiles:
                prefetch_mask(jp)
            return mask_tiles[jp]

        ht_t = [None] * NJC

        def load_ht(jc):
            sl = slice(jc * 512, (jc + 1) * 512)
            t = htpool.tile([128, KT, 512], bf16, tag="ht", name=f"ht{jc}")
            nc.sync.dma_start(out=t, in_=ht_d.ap()[:, :, sl])
            ht_t[jc] = t

        for jc in range(4):
            load_ht(jc)
        onehot_sb = singles.tile([16, 2048], f16)
        nc.sync.dma_start(out=onehot_sb, in_=oh_d.ap())
        prefetch_mask(0, split=True)
        prefetch_mask(1)
        load_ht(4)
        prefetch_mask(2)
        load_ht(5)
        load_ht(6)
        prefetch_mask(3)
        load_ht(7)

        # ---- persistent weight tiles -------------------------------------
        w1g = [singles.tile([128, 4, 2, 66], bf16, name=f"w1g{g}")
               for g in range(8)]

        def w1c(jt):
            return w1g[jt // 4][:, jt % 4]          # [128, 2, 66]

        stg = [singles.tile([128, 4, 16], f32r, name=f"stg{half}")
               for half in range(2)]
        for g in range(8):
            nc.gpsimd.memset(w1g[g][:, :, :, 64:66], 1.0)

        f_t = [singles.tile([128, 2, 16], f32, name=f"f{half}")
               for half in range(2)]
        f2_t = [singles.tile([128, 2, 16], f32, name=f"f2{half}")
                for half in range(2)]
        nf_t = [singles.tile([128, 2, 16], f32, name=f"nf{half}")
                for half in range(2)]
        hb = [singles.tile([128, RPC], bf16, tag=f"hb{h}", name=f"hb{h}")
              for h in range(2)]

        elpbig = [None, None]

        def el_zero(half):
            ep = psum.tile([128, 64], f32, tag="ps", name=f"elpbig{half}")
            nc.scalar.activation(ep, wm_sb.rearrange(
                "p a b -> p (a b)")[:, 0:64], Act.Identity, bias=0.0,
                scale=0.0)
            elpbig[half] = ep

        def el_batch(half, b):
            # j-tiles half*16+4b .. +3 (ht chunk 4*half+b)
            ep = elpbig[half]
            for jtl in range(4 * b, 4 * b + 4):
                jt = half * 16 + jtl
                jc, q = jt // 4, jt % 4
                for kt in range(KT):
                    nc.tensor.matmul(ep[:, jtl * 4:(jtl + 1) * 4],
                                     ht_t[jc][:, kt, q * 128:(q + 1) * 128],
                                     wm_sb[:, kt, 128:132],
                                     start=False, stop=(kt == KT - 1))

        def el_stage(half):
            nc.scalar.activation(
                stg[half],
                elpbig[half].rearrange("p (t q) -> p q t", q=4), Act.Copy)

        def fexp(half):
            for h in range(2):
                nc.scalar.activation(f_t[half][:, h, :],
                                     stg[half][:, 2 + h, :], Act.Exp,
                                     bias=ba_sb[:, 2 * h:2 * h + 1], scale=1.0)
                nc.scalar.activation(f2_t[half][:, h, :],
                                     stg[half][:, 2 + h, :], Act.Exp,
                                     bias=ba_sb[:, 2 * h + 1:2 * h + 2],
                                     scale=0.2)
            nc.gpsimd.tensor_scalar(
                nf_t[half].rearrange("p a b -> p (a b)"),
                f_t[half].rearrange("p a b -> p (a b)"), -1.0, None, Alu.mult)

        def prep_wh_group(g):
            # 4 jts (4g..4g+3) into one full psum bank, ONE batched copy out.
            whp = psum.tile([128, 4, 128], f32, tag="ps")
            for q2 in range(4):
                jt = 4 * g + q2
                jc, q = jt // 4, jt % 4
                for kt in range(KT):
                    nc.tensor.matmul(whp[:, q2, :],
                                     ht_t[jc][:, kt, q * 128:(q + 1) * 128],
                                     wm_sb[:, kt, 0:128],
                                     start=(q2 == 0 and kt == 0),
                                     stop=(kt == KT - 1))
            nc.scalar.activation(
                w1g[g][:, :, :, 0:64],
                whp.rearrange("p a (h o) -> p a h o", h=2),
                Act.Copy)

        # ---- front: half-0 el -> F/F2/hb; Wh groups 0..3 ------------------
        el_zero(0)
        for b in range(4):
            el_batch(0, b)
        el_stage(0)
        fexp(0)

        for h in range(2):
            trp = psum.tile([16, 128], f32r, tag="ps")
            nc.tensor.transpose(trp, stg[0][:, h, 0:16], ident_sb)
            elT = spool.tile([16, 128], f16, tag="elT")
            nc.scalar.activation(elT, trp, Act.Copy)
            for g in range(4):
                hbp = psum.tile([128, 512], f32, tag="ps")
                for tq in range(4):
                    t = g * 4 + tq
                    oh = onehot_sb[:, t * 128:(t + 1) * 128]
                    nc.tensor.matmul(hbp[:, tq * 128:(tq + 1) * 128],
                                     oh, elT, start=True, stop=True)
                nc.scalar.activation(hb[h][:, g * 512:(g + 1) * 512], hbp,
                                     Act.Exp, scale=-0.8)

        for g in range(4):
            prep_wh_group(g)
        el_zero(1)   # psum tile allocated up front; batches emitted later

        # ---- accumulators: 4 physical banks + 1 denominator bank ----------
        # Each bank region [:, k, :] is 128 wide and holds head-0 at cols
        # 0:64 and head-1 at 64:128 (the softmax denominators live in their
        # own bank denb, fed by 1-column matmuls — Ldweights/1-col matmuls
        # are nearly free, and this packing halves PSUM pressure).
        accmap = {(h, g): (g, 64 * h) for h in range(2) for g in range(4)}
        accb = [psum.tile([128, 4, 128], f32, tag="ps", name=f"acc{b}")
                for b in range(4)]
        denb = psum.tile([128, 2, 16], f32, tag="ps", name="denb")

        def acc_ap(h, g, k):
            b, off = accmap[(h, g)]
            return accb[b][:, k, off:off + 64]

        def scal(kind, jt, h):
            half, col = jt // 16, jt % 16
            t = {'f': f_t, 'f2': f2_t, 'nf': nf_t}[kind][half]
            return t[:, h, col:col + 1]

        aq = {}
        w1f = {}
        zq = {}

        def phase_a(jt):
            path = PATH_JT[jt]
            pair = apool.tile([128, 2, RPC], bf16, tag="a")
            for h in range(2):
                if path in ('AD', 'AM'):
                    nc.vector.tensor_scalar(
                        pair[:, h, :], hb[h], scal('f2', jt, h),
                        scal('f', jt, h), Alu.mult, Alu.max)
                else:
                    nc.scalar.activation(pair[:, h, :], hb[h], Act.Relu,
                                         bias=scal('nf', jt, h),
                                         scale=scal('f2', jt, h))
                    wf = singles.tile([128, 66], bf16, name=f"w1f_{jt}_{h}")
                    nc.gpsimd.tensor_scalar(wf, w1c(jt)[:, h, :],
                                            scal('f', jt, h), None, Alu.mult)
                    w1f[(jt, h)] = wf
            aq[jt] = pair

        def phase_b(jt):
            path = PATH_JT[jt]
            jp, q = jt // 2, jt % 2
            m_t = get_mask(jp)
            mb = m_t[:, q:q + 1, :].broadcast_to([128, 2, RPC])
            pair = aq.pop(jt)
            z = zpool.tile([128, 2, RPC], bf16, tag="z")
            if path in ('AD', 'CD'):
                nc.vector.tensor_tensor(z, pair, mb, Alu.mult)
            else:
                nc.gpsimd.tensor_tensor(z, pair, mb, Alu.mult)
            zq[jt] = z

        started = set()

        def consume_one(jt, h, z, g_range, stop_last):
            path = PATH_JT[jt]
            jp, q = jt // 2, jt % 2
            m_t = get_mask(jp)
            extra = path in ('CD', 'CM')
            for g in g_range:
                b, off = accmap[(h, g)]
                for k in range(4):
                    ib = g * 4 + k
                    zs = z[:, h, ib * 128:(ib + 1) * 128]
                    ms = m_t[:, q, ib * 128:(ib + 1) * 128]
                    first = b not in started
                    started.add(b)
                    nc.tensor.matmul(
                        acc_ap(h, g, k), zs, w1c(jt)[:, h, 0:64],
                        start=first,
                        stop=(stop_last and not extra))
                    dfirst = 'den' not in started
                    started.add('den')
                    nc.tensor.matmul(
                        denb[:, h, ib:ib + 1], zs, w1c(jt)[:, h, 64:65],
                        start=dfirst,
                        stop=(stop_last and not extra))
                    if extra:
                        nc.tensor.matmul(
                            acc_ap(h, g, k), ms, w1f[(jt, h)][:, 0:64],
                            start=False, stop=stop_last)
                        nc.tensor.matmul(
                            denb[:, h, ib:ib + 1], ms,
                            w1f[(jt, h)][:, 64:65],
                            start=False, stop=stop_last)

        def post_bank(h, g):
            b, off = accmap[(h, g)]
            ag = accb[b]
            dinv = spool.tile([128, 4], f32, tag="dinv")
            nc.vector.reciprocal(dinv, denb[:, h, 4 * g:4 * g + 4])
            y = spool.tile([128, 4, 64], f32, tag="y")
            for k in range(4):
                nc.vector.tensor_scalar(y[:, k, :], ag[:, k, off:off + 64],
                                        dinv[:, k:k + 1], None, Alu.mult)
            e_t = spool.tile([128, 4, 64], f32, tag="e")
            nc.scalar.activation(e_t, y, Act.Exp)
            r2 = spool.tile([128, 4, 64], f32, tag="r2")
            nc.scalar.activation(r2, e_t, Act.Relu, bias=1.0, scale=-1.0)
            r1 = spool.tile([128, 4, 64], f32, tag="r1")
            nc.gpsimd.tensor_scalar(
                r1.rearrange("p a b -> p (a b)"),
                y.rearrange("p a b -> p (a b)"), 0.0, None, Alu.max)
            o_t = spool.tile([128, 4, 64], f32, tag="o")
            nc.gpsimd.tensor_tensor(
                o_t.rearrange("p a b -> p (a b)"),
                r1.rearrange("p a b -> p (a b)"),
                r2.rearrange("p a b -> p (a b)"), Alu.subtract)
            nc.sync.dma_start(out=out_d.ap()[h, g],
                              in_=o_t.rearrange("p a b -> p (a b)"))

        # ---- steady loop ---------------------------------------------------
        for s in range(PLOOK):
            phase_a(s)
        for jt in range(JT):
            # deferred half-1 preparations, placed after their ht chunks land
            if 8 <= jt <= 11:
                el_batch(1, jt - 8)
            if jt == 12:
                el_stage(1)
                fexp(1)
            if 12 <= jt <= 15:
                prep_wh_group(jt - 8)
            if jt + PLOOK < JT:
                phase_a(jt + PLOOK)
            phase_b(jt)
            if jt // 2 + 2 < JT // 2:
                get_mask(jt // 2 + 2)
            if jt < JT - 1:
                z = zq.pop(jt)
                for h in range(2):
                    consume_one(jt, h, z, range(4), stop_last=False)
            else:
                # final jt: bank-by-bank, post fires as each bank completes
                z = zq.pop(jt)
                for h in range(2):
                    for g in range(4):
                        consume_one(jt, h, z, [g], stop_last=True)
                        post_bank(h, g)

    nc.compile()
    _prog_cache[key] = nc
    return nc


def kernel(h, mask, W, bW, a_l, a_r, bA):
    from concourse import bass_utils

    assert not np.any(np.asarray(bW)), "nonzero bW not supported"
    h = np.asarray(h, np.float32)
    mask = np.asarray(mask)
    W = np.asarray(W, np.float32)
    a_l = np.asarray(a_l, np.float32)
    a_r = np.asarray(a_r, np.float32)
    bA = np.asarray(bA, np.float32)

    nc = _build_program()

    hT = np.ascontiguousarray(h.T)                      # [F_IN, N]

    ident = np.eye(128, dtype=np.float32)
    onehot = np.zeros((16, 16 * 128), np.float16)
    for t in range(16):
        onehot[t, t * 128:(t + 1) * 128] = 1.0

    in_maps = []
    for c in range(NCORES):
        g2, r = c // 2, c % 2
        i0 = r * RPC
        heads = [2 * g2, 2 * g2 + 1]
        hT_roll = np.roll(hT, -i0, axis=1)
        ht_bf = np.ascontiguousarray(
            hT_roll.reshape(KT, 128, N).transpose(1, 0, 2)).astype(BF16)

        wmov = np.zeros((128, KT, 132), np.float32)
        for hh in range(2):
            W_ = W[heads[hh]]                           # [512, 64]
            wmov[:, :, hh * 64:(hh + 1) * 64] = \
                W_.reshape(KT, 128, 64).transpose(1, 0, 2)
            wal = (W_.astype(np.float64) @ a_l[heads[hh]].astype(np.float64))
            war = (W_.astype(np.float64) @ a_r[heads[hh]].astype(np.float64))
            wmov[:, :, 128 + hh] = wal.reshape(KT, 128).T
            wmov[:, :, 130 + hh] = war.reshape(KT, 128).T

        pack = np.zeros((128, 132), np.float32)
        pack[:, 0:128] = ident
        pack[:, 128] = bA[heads[0]]
        pack[:, 129] = 0.2 * bA[heads[0]]
        pack[:, 130] = bA[heads[1]]
        pack[:, 131] = 0.2 * bA[heads[1]]

        masklocal = np.roll(mask[i0:i0 + RPC, :], -i0, axis=1).T  # [N, RPC]
        maskt = (masklocal.astype(BF16).reshape(JT // 2, 2, 128, RPC)
                 .transpose(0, 2, 1, 3))

        in_maps.append({
            "ht": ht_bf,
            "wm": wmov.astype(BF16),
            "pack": pack,
            "oh": onehot,
            "maskt": np.ascontiguousarray(maskt),
        })

    res = bass_utils.run_bass_kernel_spmd(nc, in_maps,
                                          core_ids=list(range(NCORES)))

    out = np.empty((N, H * F_OUT), np.float32)
    for c in range(NCORES):
        g2, r = c // 2, c % 2
        i0 = r * RPC
        o = res.results[c]["out"]             # [2, 4, 128(p), 256]
        o = o.reshape(2, 4, 128, 4, F_OUT)
        o = o.transpose(0, 1, 3, 2, 4).reshape(2, RPC, F_OUT)
        for hh in range(2):
            head = 2 * g2 + hh
            out[i0:i0 + RPC, head * 64:(head + 1) * 64] = o[hh]
    return out


# revision 15
# speedup vs baseline: 1.0395x; 1.0395x over previous
# GAT layer kernel for 8 Trainium2 NeuronCores — v4 (multi-engine z-gen,
# software-pipelined front).
#
# Reference computation (per head h):
#   Wh = h @ W[h] + bW[h]                     [N, 64]
#   e[i,j] = LeakyReLU(a_l.Wh_i + a_r.Wh_j + bA, 0.2), masked, softmax over j
#   out[:, h*64:(h+1)*64] = elu(softmax(e) @ Wh)
#
# Algebraic restructure (no per-element transcendental): softmax rows are
# invariant to scaling by exp(el_i), so the unnormalized attention operand is
#   z[j,i] = mask[i,j] * max(F[j], F2[j]*Hn[i])
# with F = exp(er+bA), F2 = exp(0.2*(er+bA)), Hn = exp(-0.8*el).
#
# v4: the z elementwise work (the bottleneck) is spread over DVE, Pool and
# ACT via per-j-tile paths (both heads of a jt share one path so the mask
# multiply handles the head PAIR in one op with a broadcast mask AP):
#   AD: a2 = (hb*F2) max F on DVE (tensor_scalar dual, 4x mode) per head,
#       z = a2*mask on DVE (tensor_tensor 2x, [128,2,2048] pair)
#   AM: a2 on DVE, pair z = a2*mask on Pool (tensor_tensor Multiply)
#   CD: r = relu(F2*hb - F) on ACT (scale/bias per-partition pointers),
#       pair z_r = r*mask on DVE; the missing mask*F*Wh term is added by
#       extra PE matmuls (stationary = raw mask, moving = w1f = [F*Wh|F])
#   CM: r on ACT, pair z_r = r*mask on Pool, PE extra as in CD
# The steady matmul is reversed (stationary = z j-tile, moving = [Wh|1], 66
# cols -> 66 PE rows/matmul) and accumulators land in [i, o] layout.
#
# Front pipeline: el/Wh prep for j-tiles 16..31 depends on the second half of
# the ht DMA stream, so those preparations are EMITTED inside the steady loop
# (after the ht chunks land) to avoid head-of-line blocking every engine
# queue.  PSUM: 6 accumulator banks (head-1 g=2,3 are packed into the spare
# columns 66:132 of head-0's banks) + 1 bank for the deferred el projections
# + 1 rotating bank for Wh prep.
#
# Sharding: 8 cores = 4 head-pairs x 2 row-halves (2 heads x 2048 rows per
# core, attention over all 4096 columns); h/mask columns rolled per-core so
# own rows sit at j-tiles 0..15 (shared SPMD program).

import numpy as np
import ml_dtypes

N = 4096
F_IN = 512
F_OUT = 64
H = 8
NCORES = 8
RPC = 2048           # rows per core
KT = F_IN // 128     # 4 k-tiles
JT = N // 128        # 32 j-tiles
NJC = N // 512       # 8 ht DMA chunks
IB = RPC // 128      # 16 i-blocks
BF16 = ml_dtypes.bfloat16

# Per-jt path counts (both heads of a jt share the path).
# Pair costs: a2 2x593 DVE / relu 2x2077 ACT; mask-pair 2193 DVE / 8218 Pool;
# CD/CM add 2x16 extra PE matmuls (~0.9us).
_PJT = {
    'AD': 11,
    'AM': 6,
    'CD': 13,
    'CM': 2,
}

PLOOK = 2            # pairs of lookahead for phase-A emission


def _mk_path_table():
    order = []
    counts = dict(_PJT)
    assert sum(counts.values()) == 32
    acc = {k: 0.0 for k in counts}
    for _ in range(32):
        best = max(counts, key=lambda k: counts[k] / 32.0 * (len(order) + 1) - acc[k])
        order.append(best)
        acc[best] += 1.0
    return order


PATH_JT = _mk_path_table()

_prog_cache = {}


def _build_program():
    key = ("nc4",)
    if key in _prog_cache:
        return _prog_cache[key]
    from contextlib import ExitStack
    import concourse.tile as tile
    from concourse import bacc, mybir

    dt = mybir.dt
    f32, bf16, f16, f32r = dt.float32, dt.bfloat16, dt.float16, dt.float32r
    Alu = mybir.AluOpType
    Act = mybir.ActivationFunctionType

    nc = bacc.Bacc("TRN2", target_bir_lowering=False, debug=False,
                   num_devices=NCORES)

    ht_d = nc.dram_tensor("ht", [128, KT, N], bf16, kind="ExternalInput")
    wm_d = nc.dram_tensor("wm", [128, KT, 132], bf16, kind="ExternalInput")
    pack_d = nc.dram_tensor("pack", [128, 132], f32r, kind="ExternalInput")
    oh_d = nc.dram_tensor("oh", [16, 2048], f16, kind="ExternalInput")
    maskt_d = nc.dram_tensor("maskt", [JT // 2, 128, 2, RPC], bf16,
                             kind="ExternalInput")
    out_d = nc.dram_tensor("out", [2, 4, 128, 4 * F_OUT], f32,
                           kind="ExternalOutput")

    with tile.TileContext(nc) as tc, ExitStack() as ctx:
        singles = ctx.enter_context(tc.tile_pool(name="singles", bufs=1))
        psum = ctx.enter_context(tc.tile_pool(name="ps", bufs=8, space="PSUM"))
        mpool = ctx.enter_context(tc.tile_pool(name="mp", bufs=8))
        apool = ctx.enter_context(tc.tile_pool(name="ap", bufs=8))
        zpool = ctx.enter_context(tc.tile_pool(name="zp", bufs=9))
        htpool = ctx.enter_context(tc.tile_pool(name="ht", bufs=8))
        spool = ctx.enter_context(tc.tile_pool(name="sp", bufs=5))

        # ---- input DMA stream (one serial queue; order = priority) -------
        pack_sb = singles.tile([128, 132], f32r)
        nc.sync.dma_start(out=pack_sb, in_=pack_d.ap())
        ident_sb = pack_sb[:, 0:128]
        ba_sb = pack_sb[:, 128:132].bitcast(f32)

        wm_sb = singles.tile([128, KT, 132], bf16)
        nc.sync.dma_start(out=wm_sb, in_=wm_d.ap())

        mask_tiles = {}

        def prefetch_mask(jt):
            jp, q = jt // 2, jt % 2
            m_t = mpool.tile([128, RPC], bf16, tag="m", name=f"mpre{jt}")
            nc.sync.dma_start(out=m_t, in_=maskt_d.ap()[jp, :, q])
            mask_tiles[jt] = m_t

        def get_mask(jt):
            if jt not in mask_tiles:
                prefetch_mask(jt)
            return mask_tiles[jt]

        ht_t = [None] * NJC

        def load_ht(jc):
            sl = slice(jc * 512, (jc + 1) * 512)
            t = htpool.tile([128, KT, 512], bf16, tag="ht", name=f"ht{jc}")
            nc.sync.dma_start(out=t, in_=ht_d.ap()[:, :, sl])
            ht_t[jc] = t

        for jc in range(4):
            load_ht(jc)
        onehot_sb = singles.tile([16, 2048], f16)
        nc.sync.dma_start(out=onehot_sb, in_=oh_d.ap())
        for jt in range(4):
            prefetch_mask(jt)
        load_ht(4)
        prefetch_mask(4)
        load_ht(5)
        prefetch_mask(5)
        load_ht(6)
        prefetch_mask(6)
        load_ht(7)
        prefetch_mask(7)

        # ---- persistent weight tiles -------------------------------------
        w1g = [singles.tile([128, 4, 2, 66], bf16, name=f"w1g{g}")
               for g in range(8)]

        def w1c(jt):
            return w1g[jt // 4][:, jt % 4]          # [128, 2, 66]

        stg = [singles.tile([128, 4, 16], f32r, name=f"stg{half}")
               for half in range(2)]
        for g in range(8):
            nc.gpsimd.memset(w1g[g][:, :, :, 64:66], 1.0)

        f_t = [singles.tile([128, 2, 16], f32, name=f"f{half}")
               for half in range(2)]
        f2_t = [singles.tile([128, 2, 16], f32, name=f"f2{half}")
                for half in range(2)]
        nf_t = [singles.tile([128, 2, 16], f32, name=f"nf{half}")
                for half in range(2)]
        hb = [singles.tile([128, RPC], bf16, tag=f"hb{h}", name=f"hb{h}")
              for h in range(2)]

        elpbig = [None, None]

        def el_zero(half):
            ep = psum.tile([128, 64], f32, tag="ps", name=f"elpbig{half}")
            nc.scalar.activation(ep, wm_sb.rearrange(
                "p a b -> p (a b)")[:, 0:64], Act.Identity, bias=0.0,
                scale=0.0)
            elpbig[half] = ep

        def el_batch(half, b):
            # j-tiles half*16+4b .. +3 (ht chunk 4*half+b)
            ep = elpbig[half]
            for jtl in range(4 * b, 4 * b + 4):
                jt = half * 16 + jtl
                jc, q = jt // 4, jt % 4
                for kt in range(KT):
                    nc.tensor.matmul(ep[:, jtl * 4:(jtl + 1) * 4],
                                     ht_t[jc][:, kt, q * 128:(q + 1) * 128],
                                     wm_sb[:, kt, 128:132],
                                     start=False, stop=(kt == KT - 1))

        def el_stage(half):
            nc.scalar.activation(
                stg[half],
                elpbig[half].rearrange("p (t q) -> p q t", q=4), Act.Copy)

        def fexp(half):
            for h in range(2):
                nc.scalar.activation(f_t[half][:, h, :],
                                     stg[half][:, 2 + h, :], Act.Exp,
                                     bias=ba_sb[:, 2 * h:2 * h + 1], scale=1.0)
                nc.scalar.activation(f2_t[half][:, h, :],
                                     stg[half][:, 2 + h, :], Act.Exp,
                                     bias=ba_sb[:, 2 * h + 1:2 * h + 2],
                                     scale=0.2)
            nc.gpsimd.tensor_scalar(
                nf_t[half].rearrange("p a b -> p (a b)"),
                f_t[half].rearrange("p a b -> p (a b)"), -1.0, None, Alu.mult)

        def prep_wh_group(g):
            # 4 jts (4g..4g+3) into one full psum bank, ONE batched copy out.
            whp = psum.tile([128, 4, 128], f32, tag="ps")
            for q2 in range(4):
                jt = 4 * g + q2
                jc, q = jt // 4, jt % 4
                for kt in range(KT):
                    nc.tensor.matmul(whp[:, q2, :],
                                     ht_t[jc][:, kt, q * 128:(q + 1) * 128],
                                     wm_sb[:, kt, 0:128],
                                     start=(q2 == 0 and kt == 0),
                                     stop=(kt == KT - 1))
            nc.scalar.activation(
                w1g[g][:, :, :, 0:64],
                whp.rearrange("p a (h o) -> p a h o", h=2),
                Act.Copy)

        # ---- front: half-0 el -> F/F2/hb; Wh groups 0..3 ------------------
        el_zero(0)
        for b in range(4):
            el_batch(0, b)
        el_stage(0)
        fexp(0)

        for h in range(2):
            trp = psum.tile([16, 128], f32r, tag="ps")
            nc.tensor.transpose(trp, stg[0][:, h, 0:16], ident_sb)
            elT = spool.tile([16, 128], f16, tag="elT")
            nc.scalar.activation(elT, trp, Act.Copy)
            for g in range(4):
                hbp = psum.tile([128, 512], f32, tag="ps")
                for tq in range(4):
                    t = g * 4 + tq
                    oh = onehot_sb[:, t * 128:(t + 1) * 128]
                    nc.tensor.matmul(hbp[:, tq * 128:(tq + 1) * 128],
                                     oh, elT, start=True, stop=True)
                nc.scalar.activation(hb[h][:, g * 512:(g + 1) * 512], hbp,
                                     Act.Exp, scale=-0.8)

        for g in range(4):
            prep_wh_group(g)
        el_zero(1)   # psum tile allocated up front; batches emitted later

        # ---- accumulators: 4 physical banks + 1 denominator bank ----------
        # Each bank region [:, k, :] is 128 wide and holds head-0 at cols
        # 0:64 and head-1 at 64:128 (the softmax denominators live in their
        # own bank denb, fed by 1-column matmuls — Ldweights/1-col matmuls
        # are nearly free, and this packing halves PSUM pressure).
        accmap = {(h, g): (g, 64 * h) for h in range(2) for g in range(4)}
        accb = [psum.tile([128, 4, 128], f32, tag="ps", name=f"acc{b}")
                for b in range(4)]
        denb = psum.tile([128, 2, 16], f32, tag="ps", name="denb")

        def acc_ap(h, g, k):
            b, off = accmap[(h, g)]
            return accb[b][:, k, off:off + 64]

        def scal(kind, jt, h):
            half, col = jt // 16, jt % 16
            t = {'f': f_t, 'f2': f2_t, 'nf': nf_t}[kind][half]
            return t[:, h, col:col + 1]

        aq = {}
        w1f = {}
        zq = {}

        def phase_a(jt, h):
            path = PATH_JT[jt]
            a = apool.tile([128, RPC], bf16, tag="a")
            if path in ('AD', 'AM'):
                nc.vector.tensor_scalar(
                    a, hb[h], scal('f2', jt, h),
                    scal('f', jt, h), Alu.mult, Alu.max)
            else:
                nc.scalar.activation(a, hb[h], Act.Relu,
                                     bias=scal('nf', jt, h),
                                     scale=scal('f2', jt, h))
                wf = singles.tile([128, 66], bf16, name=f"w1f_{jt}_{h}")
                nc.gpsimd.tensor_scalar(wf, w1c(jt)[:, h, :],
                                        scal('f', jt, h), None, Alu.mult)
                w1f[(jt, h)] = wf
            aq[(jt, h)] = a

        def phase_b(jt, h):
            path = PATH_JT[jt]
            m_t = get_mask(jt)
            a = aq.pop((jt, h))
            z = zpool.tile([128, RPC], bf16, tag="z")
            if path in ('AD', 'CD'):
                nc.vector.tensor_tensor(z, a, m_t, Alu.mult)
            else:
                nc.gpsimd.tensor_tensor(z, a, m_t, Alu.mult)
            zq[(jt, h)] = z

        started = set()

        def consume_one(jt, h, z, g_range, stop_last):
            path = PATH_JT[jt]
            m_t = get_mask(jt)
            extra = path in ('CD', 'CM')
            for g in g_range:
                b, off = accmap[(h, g)]
                for k in range(4):
                    ib = g * 4 + k
                    zs = z[:, ib * 128:(ib + 1) * 128]
                    ms = m_t[:, ib * 128:(ib + 1) * 128]
                    first = b not in started
                    started.add(b)
                    nc.tensor.matmul(
                        acc_ap(h, g, k), zs, w1c(jt)[:, h, 0:64],
                        start=first,
                        stop=(stop_last and not extra))
                    dfirst = 'den' not in started
                    started.add('den')
                    nc.tensor.matmul(
                        denb[:, h, ib:ib + 1], zs, w1c(jt)[:, h, 64:65],
                        start=dfirst,
                        stop=(stop_last and not extra))
                    if extra:
                        nc.tensor.matmul(
                            acc_ap(h, g, k), ms, w1f[(jt, h)][:, 0:64],
                            start=False, stop=stop_last)
                        nc.tensor.matmul(
                            denb[:, h, ib:ib + 1], ms,
                            w1f[(jt, h)][:, 64:65],
                            start=False, stop=stop_last)

        def post_bank(h, g):
            b, off = accmap[(h, g)]
            ag = accb[b]
            dinv = spool.tile([128, 4], f32, tag="dinv")
            nc.vector.reciprocal(dinv, denb[:, h, 4 * g:4 * g + 4])
            y = spool.tile([128, 4, 64], f32, tag="y")
            for k in range(4):
                nc.vector.tensor_scalar(y[:, k, :], ag[:, k, off:off + 64],
                                        dinv[:, k:k + 1], None, Alu.mult)
            e_t = spool.tile([128, 4, 64], f32, tag="e")
            nc.scalar.activation(e_t, y, Act.Exp)
            r2 = spool.tile([128, 4, 64], f32, tag="r2")
            nc.scalar.activation(r2, e_t, Act.Relu, bias=1.0, scale=-1.0)
            r1 = spool.tile([128, 4, 64], f32, tag="r1")
            nc.gpsimd.tensor_scalar(
                r1.rearrange("p a b -> p (a b)"),
                y.rearrange("p a b -> p (a b)"), 0.0, None, Alu.max)
            o_t = spool.tile([128, 4, 64], f32, tag="o")
            nc.gpsimd.tensor_tensor(
                o_t.rearrange("p a b -> p (a b)"),
                r1.rearrange("p a b -> p (a b)"),
                r2.rearrange("p a b -> p (a b)"), Alu.subtract)
            nc.sync.dma_start(out=out_d.ap()[h, g],
                              in_=o_t.rearrange("p a b -> p (a b)"))

        # ---- steady loop ---------------------------------------------------
        for s in range(PLOOK):
            phase_a(s, 0)
            phase_a(s, 1)
        for jt in range(JT):
            # deferred half-1 preparations (their ht chunks land by ~t25;
            # the PE reaches these queue positions later than that, so no
            # head-of-line stall)
            if 10 <= jt <= 13:
                el_batch(1, jt - 10)
            if jt == 14:
                el_stage(1)
                fexp(1)
            if 14 <= jt <= 17:
                prep_wh_group(jt - 10)
            if jt + PLOOK < JT:
                phase_a(jt + PLOOK, 0)
                phase_a(jt + PLOOK, 1)
            if jt + 4 < JT:
                get_mask(jt + 4)
            if jt < JT - 1:
                for h in range(2):
                    phase_b(jt, h)
                    z = zq.pop((jt, h))
                    consume_one(jt, h, z, range(4), stop_last=False)
            else:
                # final jt: bank-by-bank, post fires as each bank completes
                for h in range(2):
                    phase_b(jt, h)
                for h in range(2):
                    z = zq.pop((jt, h))
                    for g in range(4):
                        consume_one(jt, h, z, [g], stop_last=True)
                        post_bank(h, g)

    nc.compile()
    _prog_cache[key] = nc
    return nc


def kernel(h, mask, W, bW, a_l, a_r, bA):
    from concourse import bass_utils

    assert not np.any(np.asarray(bW)), "nonzero bW not supported"
    h = np.asarray(h, np.float32)
    mask = np.asarray(mask)
    W = np.asarray(W, np.float32)
    a_l = np.asarray(a_l, np.float32)
    a_r = np.asarray(a_r, np.float32)
    bA = np.asarray(bA, np.float32)

    nc = _build_program()

    hT = np.ascontiguousarray(h.T)                      # [F_IN, N]

    ident = np.eye(128, dtype=np.float32)
    onehot = np.zeros((16, 16 * 128), np.float16)
    for t in range(16):
        onehot[t, t * 128:(t + 1) * 128] = 1.0

    in_maps = []
    for c in range(NCORES):
        g2, r = c // 2, c % 2
        i0 = r * RPC
        heads = [2 * g2, 2 * g2 + 1]
        hT_roll = np.roll(hT, -i0, axis=1)
        ht_bf = np.ascontiguousarray(
            hT_roll.reshape(KT, 128, N).transpose(1, 0, 2)).astype(BF16)

        wmov = np.zeros((128, KT, 132), np.float32)
        for hh in range(2):
            W_ = W[heads[hh]]                           # [512, 64]
            wmov[:, :, hh * 64:(hh + 1) * 64] = \
                W_.reshape(KT, 128, 64).transpose(1, 0, 2)
            wal = (W_.astype(np.float64) @ a_l[heads[hh]].astype(np.float64))
            war = (W_.astype(np.float64) @ a_r[heads[hh]].astype(np.float64))
            wmov[:, :, 128 + hh] = wal.reshape(KT, 128).T
            wmov[:, :, 130 + hh] = war.reshape(KT, 128).T

        pack = np.zeros((128, 132), np.float32)
        pack[:, 0:128] = ident
        pack[:, 128] = bA[heads[0]]
        pack[:, 129] = 0.2 * bA[heads[0]]
        pack[:, 130] = bA[heads[1]]
        pack[:, 131] = 0.2 * bA[heads[1]]

        masklocal = np.roll(mask[i0:i0 + RPC, :], -i0, axis=1).T  # [N, RPC]
        maskt = (masklocal.astype(BF16).reshape(JT // 2, 2, 128, RPC)
                 .transpose(0, 2, 1, 3))

        in_maps.append({
            "ht": ht_bf,
            "wm": wmov.astype(BF16),
            "pack": pack,
            "oh": onehot,
            "maskt": np.ascontiguousarray(maskt),
        })

    res = bass_utils.run_bass_kernel_spmd(nc, in_maps,
                                          core_ids=list(range(NCORES)))

    out = np.empty((N, H * F_OUT), np.float32)
    for c in range(NCORES):
        g2, r = c // 2, c % 2
        i0 = r * RPC
        o = res.results[c]["out"]             # [2, 4, 128(p), 256]
        o = o.reshape(2, 4, 128, 4, F_OUT)
        o = o.transpose(0, 1, 3, 2, 4).reshape(2, RPC, F_OUT)
        for hh in range(2):
            head = 2 * g2 + hh
            out[i0:i0 + RPC, head * 64:(head + 1) * 64] = o[hh]
    return out
